# revision 45
# baseline (speedup 1.0000x reference)
"""HAR 6-channel 2-layer LSTM encoder bank on TRN2 (Bass/Tile, 8 cores).

Formulation
-----------
All 6 LSTM cells (2 layers x 3 encoders; layer 1 pipelined one step behind
layer 0) fuse into one 22x72 gate projection per timestep (21 state rows +
a ones-row carrying the biases).  All four gate groups go through a single
sigmoid (the tanh gate is pre-scaled by 2; tanh(x) = 2*sigmoid(2x)-1 is
fixed up algebraically in the c-update).

Parallelization: batch (2048 folded streams) is sharded over 8 cores; on
each core the T=2048 sequence is cut into C chunks processed in parallel
as extra batch, each warm-started W steps early (LSTM forget gates make
the state contraction ~e^-0.5/step; validated rel-err ~2e-3 in fp16).

Per-core layout is sequence-major: [128 vseq lanes, features].  Per step:
DVE 32x32 block-transpose rebuilds the [22, 32]-per-subblock stationary,
4 tile-positioned concurrent matmuls per 128-lane block compute gates into
PSUM, one fused Sigmoid on ScalarE, STT-fused cell update on VectorE.
"""
import os
import numpy as np

B, T = 1024, 2048
NCORES = 8

# device-kernel tunables (full problem)
CFG_FULL = dict(spc=256, t=T, c=16, w=24, ngrp=4)


def _derive(cfg):
    spc, t, c, w, ngrp = cfg['spc'], cfg['t'], cfg['c'], cfg['w'], cfg['ngrp']
    L = t // c
    nsteps = L + w + 1
    halves = spc // 128
    nblk = spc * c // 128
    mpg = nblk // ngrp
    assert spc % 128 == 0 and t % c == 0 and nblk % ngrp == 0
    return L, nsteps, halves, nblk, mpg


def _pack_weights(inp):
    """wfull22 [22, 72]: rows 0:9 h0(e,k), 9:18 h1(e,k), 18:21 x, 21 bias.

    gate cols: gt*18 + stream*3 + k; gt in (i,f,o,g); stream = layer*3+e.
    g-gate (gt=3) pre-scaled by 2 for the single-sigmoid trick.
    """
    torch_off = {'i': 0, 'f': 3, 'g': 6, 'o': 9}
    wfull = np.zeros((22, 72), np.float32)
    for gi, gname in enumerate(['i', 'f', 'o', 'g']):
        toff = torch_off[gname]
        mul = 2.0 if gname == 'g' else 1.0
        for stream in range(6):
            layer, e = stream // 3, stream % 3
            for k in range(3):
                col = gi * 18 + stream * 3 + k
                if layer == 0:
                    wfull[18:21, col] = mul * inp['W_ih0'][e, toff + k, :]
                    wfull[3 * e:3 * e + 3, col] = mul * inp['W_hh0'][e, toff + k, :]
                    wfull[21, col] = mul * (inp['b_ih0'][e, toff + k]
                                            + inp['b_hh0'][e, toff + k])
                else:
                    wfull[3 * e:3 * e + 3, col] = mul * inp['W_ih1'][e, toff + k, :]
                    wfull[9 + 3 * e:12 + 3 * e, col] = mul * inp['W_hh1'][e, toff + k, :]
                    wfull[21, col] = mul * (inp['b_ih1'][e, toff + k]
                                            + inp['b_hh1'][e, toff + k])
    wrep = np.zeros((128, 72), np.float16)
    for i in range(4):
        wrep[32 * i:32 * i + 22] = wfull.astype(np.float16)
    return wrep


def _build_nc(cfg):
    import concourse.bass as bass
    import concourse.bacc as bacc
    import concourse.tile as tile
    from concourse import mybir
    from contextlib import ExitStack

    spc, t, c, w, ngrp = cfg['spc'], cfg['t'], cfg['c'], cfg['w'], cfg['ngrp']
    L, nsteps, halves, nblk, mpg = _derive(cfg)
    f32, f16 = mybir.dt.float32, mybir.dt.float16
    AF = mybir.ActivationFunctionType
    OP = mybir.AluOpType

    nc = bacc.Bacc("TRN2", target_bir_lowering=False, debug=False,
                   num_devices=NCORES)
    xs_ext = nc.declare_dram_parameter("xs", [spc, 3, t], f16, isOutput=False)
    wrep_ext = nc.declare_dram_parameter("wrep", [128, 72], f16, isOutput=False)
    s9_ext = nc.declare_dram_parameter("s9", [9 * t], f16, isOutput=False)
    b9_ext = nc.declare_dram_parameter("b9", [9 * t], f16, isOutput=False)
    out_ext = nc.declare_dram_parameter("out", [spc, 3, t], f32, isOutput=True)

    # block b <-> (chunk ci = b//halves, half = b%halves)
    def blk_ci(b):
        return b // halves

    def blk_t0(b):
        ci = blk_ci(b)
        return 0 if ci == 0 else ci * L - w

    def blk_off(b):
        return 0 if blk_ci(b) == 0 else w

    ctx = ExitStack()
    with tile.TileContext(nc) as tc:
        pools = {}
        for nm, bufs, space in [
            ("const", 1, "SBUF"), ("state", 1, "SBUF"), ("step", 3, "SBUF"),
            ("epi", 3, "SBUF"), ("outb", 4, "SBUF"), ("bn", 3, "SBUF"),
        ]:
            pools[nm] = ctx.enter_context(tc.tile_pool(name=nm, bufs=bufs, space=space))
        psum_pools = [ctx.enter_context(tc.tile_pool(name=f"ps{g}", bufs=1, space="PSUM"))
                      for g in range(ngrp)]

        const, state, step, epi = pools["const"], pools["state"], pools["step"], pools["epi"]

        wrep_t = const.tile([128, 72], f16, name="wrep_t")
        nc.sync.dma_start(out=wrep_t[:, :], in_=wrep_ext[:, :])

        x_stage = const.tile([128, nblk * 3 * nsteps], f16, name="x_stage")
        nc.vector.memset(x_stage[:, :], 0.0)
        xsv = x_stage[:, :].rearrange("p (b c2 s) -> p b c2 s", b=nblk, c2=3)
        for b in range(nblk):
            t0, half = blk_t0(b), b % halves
            ln = min(nsteps, t - t0)
            nc.sync.dma_start(
                out=xsv[:, b, :, 0:ln],
                in_=xs_ext[half * 128:(half + 1) * 128, :, t0:t0 + ln])

        hist = const.tile([128, nblk * L * 9], f16, name="hist")
        histv = hist[:, :].rearrange("p (b t2 j) -> p b t2 j", b=nblk, j=9)

        # one shared hin tile (so cross-group merged ops can read one AP),
        # per-group slices of it
        hin_all = state.tile([128, nblk * 32], f16, name="hin_all")
        nc.vector.memset(hin_all[:, :], 0.0)
        hinv_all = hin_all[:, :].rearrange("p (b q) -> p b q", q=32)
        nc.vector.memset(hinv_all[:, :, 21:22], 1.0)     # bias ones-row
        hin, hinv, ct, ctv, psum = [], [], [], [], []
        for g in range(ngrp):
            h = hin_all[:, g * mpg * 32:(g + 1) * mpg * 32]
            hv = hinv_all[:, g * mpg:(g + 1) * mpg, :]
            cc = state.tile([128, mpg * 18], f16, name=f"c{g}")
            nc.vector.memset(cc[:, :], 0.0)
            ccv = cc[:, :].rearrange("p (b j) -> p b j", j=18)
            ps = psum_pools[g].tile([128, mpg * 128], f32, name=f"psum{g}")
            hin.append(h); hinv.append(hv); ct.append(cc); ctv.append(ccv)
            psum.append(ps)

        # collapse the staging DMAs/memsets into one sync point so the first
        # loop instructions don't exceed the per-instruction sync-wait limit
        tc.strict_bb_all_engine_barrier()

        # ---------------- recurrence ----------------
        for tl in range(nsteps):
            for g in range(ngrp):
                b0 = g * mpg
                hv, cc, ccv, ps = hinv[g], ct[g], ctv[g], psum[g]
                # x for this step into hin cols 18:21
                nc.vector.tensor_copy(hv[:, :, 18:21], xsv[:, b0:b0 + mpg, :, tl])
                # transpose -> stationary layout
                htT = step.tile([128, mpg * 32], f16, name=f"htT{g}", tag=f"htT{g}")
                nc.vector.transpose(htT[:, :], hin[g][:, :])
                # gates
                psv = ps[:, :].rearrange("p (b q) -> p b q", q=128)
                for bb in range(mpg):
                    for i in range(4):
                        nc.tensor.matmul(
                            out=psv[32 * i:32 * i + 32, bb, 0:72],
                            lhsT=htT[32 * i:32 * i + 22, 32 * bb:32 * bb + 32],
                            rhs=wrep_t[32 * i:32 * i + 22, :],
                            start=True, stop=True,
                            tile_position=(32 * i, 32 * i))
                sg = step.tile([128, mpg * 72], f16, name=f"sg{g}", tag=f"sg{g}")
                sgv = sg[:, :].rearrange("p (b gt j) -> p b gt j", gt=4, j=18)
                nc.scalar.activation(
                    sgv[:, :, :, :], psv[:, :, 0:72].rearrange("p b (gt j) -> p b gt j", gt=4),
                    AF.Sigmoid)
                # cell update
                u = step.tile([128, mpg * 18], f16, name=f"u{g}", tag=f"u{g}")
                uv = u[:, :].rearrange("p (b j) -> p b j", j=18)
                nc.vector.scalar_tensor_tensor(
                    uv[:, :, :], sgv[:, :, 3, :], 0.5, sgv[:, :, 0, :],
                    op0=OP.subtract, op1=OP.mult)
                cf = step.tile([128, mpg * 18], f16, name=f"cf{g}", tag=f"cf{g}")
                cfv = cf[:, :].rearrange("p (b j) -> p b j", j=18)
                # runs on GPSIMD concurrently with the u-op on DVE
                nc.gpsimd.tensor_mul(cfv[:, :, :], ccv[:, :, :], sgv[:, :, 1, :])
                nc.vector.scalar_tensor_tensor(
                    ccv[:, :, :], uv[:, :, :], 2.0, cfv[:, :, :],
                    op0=OP.mult, op1=OP.add)
                th = step.tile([128, mpg * 18], f16, name=f"th{g}", tag=f"th{g}")
                thv = th[:, :].rearrange("p (b j) -> p b j", j=18)
                nc.scalar.activation(th[:, :], cc[:, :], AF.Tanh)
                nc.vector.tensor_mul(hv[:, :, 0:18], thv[:, :, :], sgv[:, :, 2, :])
                if tl == 0:
                    # layer-1 stream starts one step later
                    nc.vector.memset(hv[:, :, 9:18], 0.0)
                    nc.vector.memset(ccv[:, :, 9:18], 0.0)
            if tl > 0:
                # store h1 for blocks whose local output index is valid;
                # merged across groups and run on the (otherwise idle)
                # GPSIMD engine — off the recurrence critical path
                runs = []  # (bstart, bend, tt)
                cur = None
                for b in range(nblk):
                    tt = tl - 1 - blk_off(b)
                    key = tt if 0 <= tt < L else None
                    if cur is None or key != cur[2]:
                        if cur is not None and cur[2] is not None:
                            runs.append(cur)
                        cur = [b, b + 1, key]
                    else:
                        cur[1] = b + 1
                if cur is not None and cur[2] is not None:
                    runs.append(cur)
                for (bs, be, tt) in runs:
                    nc.gpsimd.tensor_copy(
                        histv[:, bs:be, tt, :],
                        hinv_all[:, bs:be, 9:18])

        # ---------------- epilogue: BN affine + relu + mean over encoders ----
        bnp = pools["bn"]
        for b in range(nblk):
            ci, half = blk_ci(b), b % halves
            hb = histv[:, b, :, :]                       # [128, L, 9]
            s9t = bnp.tile([128, L * 9], f16, name="s9t", tag="s9t")
            b9t = bnp.tile([128, L * 9], f16, name="b9t", tag="b9t")
            nc.sync.dma_start(
                out=s9t[:, :],
                in_=s9_ext[ci * L * 9:(ci + 1) * L * 9].unsqueeze(0).broadcast_to([128, L * 9]))
            nc.sync.dma_start(
                out=b9t[:, :],
                in_=b9_ext[ci * L * 9:(ci + 1) * L * 9].unsqueeze(0).broadcast_to([128, L * 9]))
            m1 = epi.tile([128, L * 9], f16, name="m1", tag="m1")
            m1v = m1[:, :].rearrange("p (t2 j) -> p t2 j", j=9)
            nc.gpsimd.tensor_mul(m1[:, :], hb.rearrange("p t2 j -> p (t2 j)"), s9t[:, :])
            z = epi.tile([128, L * 9], f16, name="z", tag="z")
            nc.vector.tensor_add(z[:, :], m1[:, :], b9t[:, :])
            z2 = epi.tile([128, L * 9], f16, name="z2", tag="z2")
            # relu on DVE (tensor_scalar 4x mode on contiguous fp16) — keeps
            # the epilogue off the busier ScalarE
            nc.vector.tensor_scalar_max(z2[:, :], z[:, :], 0.0)
            zv = z2[:, :].rearrange("p (t2 e k) -> p t2 e k", e=3, k=3)
            s1 = epi.tile([128, L * 3], f16, name="s1", tag="s1")
            s1v = s1[:, :].rearrange("p (t2 k) -> p t2 k", k=3)
            nc.gpsimd.tensor_add(s1v[:, :, :], zv[:, :, 0, :], zv[:, :, 1, :])
            ob = pools["outb"].tile([128, 3 * L], f32, name="ob", tag="ob")
            obv = ob[:, :].rearrange("p (k t2) -> p t2 k", k=3)
            nc.gpsimd.tensor_add(obv[:, :, :], s1v[:, :, :], zv[:, :, 2, :])
            nc.sync.dma_start(
                out=out_ext[half * 128:(half + 1) * 128, :, ci * L:(ci + 1) * L],
                in_=ob[:, :].rearrange("p (k t2) -> p k t2", k=3))
        ctx.close()
    nc.compile()   # bacc lowering: splits multi-sem waits, regalloc, fusion
    return nc


_BUILT = {}


def _get_built(cfg):
    key = tuple(sorted(cfg.items()))
    if key not in _BUILT:
        _BUILT[key] = _build_nc(cfg)
    return _BUILT[key]


def _host_pack(inp, cfg):
    """Build per-core input maps from full inputs."""
    spc, t = cfg['spc'], cfg['t']
    x = np.asarray(inp['x'], np.float32)
    wrep = _pack_weights({k: np.asarray(v, np.float32) for k, v in inp.items()
                          if k.startswith(('W_', 'b_'))})
    s = (np.asarray(inp['bn_gamma'], np.float32)
         / np.sqrt(np.asarray(inp['bn_var'], np.float32) + 1e-5))
    s3 = (s / 3.0).astype(np.float32)
    b3 = ((np.asarray(inp['bn_beta'], np.float32)
           - np.asarray(inp['bn_mean'], np.float32) * s) / 3.0).astype(np.float32)
    s9 = np.repeat(s3, 9).astype(np.float16)
    b9 = np.repeat(b3, 9).astype(np.float16)
    # stream s: 0..B-1 acc (ch 0:3), B..2B-1 gyr (ch 3:6)
    xs_all = np.concatenate([x[:, 0:3, :], x[:, 3:6, :]], axis=0).astype(np.float16)
    ncores = (2 * B) // spc
    in_maps = []
    for k in range(ncores):
        in_maps.append({
            "xs": np.ascontiguousarray(xs_all[k * spc:(k + 1) * spc]),
            "wrep": wrep, "s9": s9, "b9": b9,
        })
    return in_maps


def _unpack(results, cfg):
    spc, t = cfg['spc'], cfg['t']
    ncores = (2 * B) // spc
    out = np.empty((B, 6, T), np.float32)
    for k in range(ncores):
        r = np.asarray(results[k]["out"], np.float32)   # [spc, 3, t]
        s0 = k * spc
        for j in range(spc):
            s = s0 + j
            if s < B:
                out[s, 0:3, :] = r[j]
            else:
                out[s - B, 3:6, :] = r[j]
    return out


def _run(inp, trace=False):
    from concourse.bass_utils import run_bass_kernel_spmd
    cfg = CFG_FULL
    nc = _get_built(cfg)
    in_maps = _host_pack(inp, cfg)
    core_ids = list(range(len(in_maps)))
    res = run_bass_kernel_spmd(nc, in_maps, core_ids, trace=trace)
    out = _unpack(res.results, cfg)
    return out, res


def kernel(**inputs):
    out, _ = _run(inputs, trace=False)
    return out


# revision 50
# speedup vs baseline: 1.0042x; 1.0042x over previous
"""HAR 6-channel 2-layer LSTM encoder bank on TRN2 (Bass/Tile, 8 cores).

Formulation
-----------
All 6 LSTM cells (2 layers x 3 encoders; layer 1 pipelined one step behind
layer 0) fuse into one 22x72 gate projection per timestep (21 state rows +
a ones-row carrying the biases).  All four gate groups go through a single
sigmoid (the tanh gate is pre-scaled by 2; tanh(x) = 2*sigmoid(2x)-1 is
fixed up algebraically in the c-update).

Parallelization: batch (2048 folded streams) is sharded over 8 cores; on
each core the T=2048 sequence is cut into C chunks processed in parallel
as extra batch, each warm-started W steps early (LSTM forget gates make
the state contraction ~e^-0.5/step; validated rel-err ~2e-3 in fp16).

Per-core layout is sequence-major: [128 vseq lanes, features].  Per step:
DVE 32x32 block-transpose rebuilds the [22, 32]-per-subblock stationary,
4 tile-positioned concurrent matmuls per 128-lane block compute gates into
PSUM, one fused Sigmoid on ScalarE, STT-fused cell update on VectorE.
"""
import os
import numpy as np

B, T = 1024, 2048
NCORES = 8

# device-kernel tunables (full problem)
CFG_FULL = dict(spc=256, t=T, c=16, w=24, ngrp=4)


def _derive(cfg):
    spc, t, c, w, ngrp = cfg['spc'], cfg['t'], cfg['c'], cfg['w'], cfg['ngrp']
    L = t // c
    nsteps = L + w + 1
    halves = spc // 128
    nblk = spc * c // 128
    mpg = nblk // ngrp
    assert spc % 128 == 0 and t % c == 0 and nblk % ngrp == 0
    return L, nsteps, halves, nblk, mpg


def _pack_weights(inp):
    """wfull22 [22, 72]: rows 0:9 h0(e,k), 9:18 h1(e,k), 18:21 x, 21 bias.

    gate cols: gt*18 + stream*3 + k; gt in (i,f,o,g); stream = layer*3+e.
    g-gate (gt=3) pre-scaled by 2 for the single-sigmoid trick.
    """
    torch_off = {'i': 0, 'f': 3, 'g': 6, 'o': 9}
    wfull = np.zeros((22, 72), np.float32)
    for gi, gname in enumerate(['i', 'f', 'o', 'g']):
        toff = torch_off[gname]
        mul = 2.0 if gname == 'g' else 1.0
        for stream in range(6):
            layer, e = stream // 3, stream % 3
            for k in range(3):
                col = gi * 18 + stream * 3 + k
                if layer == 0:
                    wfull[18:21, col] = mul * inp['W_ih0'][e, toff + k, :]
                    wfull[3 * e:3 * e + 3, col] = mul * inp['W_hh0'][e, toff + k, :]
                    wfull[21, col] = mul * (inp['b_ih0'][e, toff + k]
                                            + inp['b_hh0'][e, toff + k])
                else:
                    wfull[3 * e:3 * e + 3, col] = mul * inp['W_ih1'][e, toff + k, :]
                    wfull[9 + 3 * e:12 + 3 * e, col] = mul * inp['W_hh1'][e, toff + k, :]
                    wfull[21, col] = mul * (inp['b_ih1'][e, toff + k]
                                            + inp['b_hh1'][e, toff + k])
    wrep = np.zeros((128, 72), np.float16)
    for i in range(4):
        wrep[32 * i:32 * i + 22] = wfull.astype(np.float16)
    return wrep


def _build_nc(cfg):
    import concourse.bass as bass
    import concourse.bacc as bacc
    import concourse.tile as tile
    from concourse import mybir
    from contextlib import ExitStack

    spc, t, c, w, ngrp = cfg['spc'], cfg['t'], cfg['c'], cfg['w'], cfg['ngrp']
    L, nsteps, halves, nblk, mpg = _derive(cfg)
    f32, f16 = mybir.dt.float32, mybir.dt.float16
    AF = mybir.ActivationFunctionType
    OP = mybir.AluOpType

    nc = bacc.Bacc("TRN2", target_bir_lowering=False, debug=False,
                   num_devices=NCORES)
    xs_ext = nc.declare_dram_parameter("xs", [spc, 3, t], f16, isOutput=False)
    wrep_ext = nc.declare_dram_parameter("wrep", [128, 72], f16, isOutput=False)
    s9_ext = nc.declare_dram_parameter("s9", [9 * t], f16, isOutput=False)
    b9_ext = nc.declare_dram_parameter("b9", [9 * t], f16, isOutput=False)
    out_ext = nc.declare_dram_parameter("out", [spc, 3, t], f32, isOutput=True)

    # block b <-> (chunk ci = b//halves, half = b%halves)
    def blk_ci(b):
        return b // halves

    def blk_t0(b):
        ci = blk_ci(b)
        return 0 if ci == 0 else ci * L - w

    def blk_off(b):
        return 0 if blk_ci(b) == 0 else w

    ctx = ExitStack()
    with tile.TileContext(nc) as tc:
        pools = {}
        for nm, bufs, space in [
            ("const", 1, "SBUF"), ("state", 1, "SBUF"), ("step", 3, "SBUF"),
            ("epi", 3, "SBUF"), ("outb", 4, "SBUF"), ("bn", 3, "SBUF"),
        ]:
            pools[nm] = ctx.enter_context(tc.tile_pool(name=nm, bufs=bufs, space=space))
        psum_pools = [ctx.enter_context(tc.tile_pool(name=f"ps{g}", bufs=1, space="PSUM"))
                      for g in range(ngrp)]

        const, state, step, epi = pools["const"], pools["state"], pools["step"], pools["epi"]

        wrep_t = const.tile([128, 72], f16, name="wrep_t")
        nc.sync.dma_start(out=wrep_t[:, :], in_=wrep_ext[:, :])

        x_stage = const.tile([128, nblk * 3 * nsteps], f16, name="x_stage")
        nc.vector.memset(x_stage[:, :], 0.0)
        xsv = x_stage[:, :].rearrange("p (b c2 s) -> p b c2 s", b=nblk, c2=3)
        for b in range(nblk):
            t0, half = blk_t0(b), b % halves
            ln = min(nsteps, t - t0)
            nc.sync.dma_start(
                out=xsv[:, b, :, 0:ln],
                in_=xs_ext[half * 128:(half + 1) * 128, :, t0:t0 + ln])

        hist = const.tile([128, nblk * L * 9], f16, name="hist")
        histv = hist[:, :].rearrange("p (b t2 j) -> p b t2 j", b=nblk, j=9)

        # one shared hin tile (so cross-group merged ops can read one AP),
        # per-group slices of it
        hin_all = state.tile([128, nblk * 32], f16, name="hin_all")
        nc.vector.memset(hin_all[:, :], 0.0)
        hinv_all = hin_all[:, :].rearrange("p (b q) -> p b q", q=32)
        nc.vector.memset(hinv_all[:, :, 21:22], 1.0)     # bias ones-row
        hin, hinv, ct, ctv, psum = [], [], [], [], []
        for g in range(ngrp):
            h = hin_all[:, g * mpg * 32:(g + 1) * mpg * 32]
            hv = hinv_all[:, g * mpg:(g + 1) * mpg, :]
            cc = state.tile([128, mpg * 18], f16, name=f"c{g}")
            nc.vector.memset(cc[:, :], 0.0)
            ccv = cc[:, :].rearrange("p (b j) -> p b j", j=18)
            ps = psum_pools[g].tile([128, mpg * 128], f32, name=f"psum{g}")
            hin.append(h); hinv.append(hv); ct.append(cc); ctv.append(ccv)
            psum.append(ps)

        # collapse the staging DMAs/memsets into one sync point so the first
        # loop instructions don't exceed the per-instruction sync-wait limit
        tc.strict_bb_all_engine_barrier()

        # ---------------- recurrence ----------------
        for tl in range(nsteps):
            for g in range(ngrp):
                b0 = g * mpg
                hv, cc, ccv, ps = hinv[g], ct[g], ctv[g], psum[g]
                # x for this step into hin cols 18:21
                nc.vector.tensor_copy(hv[:, :, 18:21], xsv[:, b0:b0 + mpg, :, tl])
                # transpose -> stationary layout
                htT = step.tile([128, mpg * 32], f16, name=f"htT{g}", tag=f"htT{g}")
                nc.vector.transpose(htT[:, :], hin[g][:, :])
                # gates
                psv = ps[:, :].rearrange("p (b q) -> p b q", q=128)
                for bb in range(mpg):
                    for i in range(4):
                        nc.tensor.matmul(
                            out=psv[32 * i:32 * i + 32, bb, 0:72],
                            lhsT=htT[32 * i:32 * i + 22, 32 * bb:32 * bb + 32],
                            rhs=wrep_t[32 * i:32 * i + 22, :],
                            start=True, stop=True,
                            tile_position=(32 * i, 32 * i))
                sg = step.tile([128, mpg * 72], f16, name=f"sg{g}", tag=f"sg{g}")
                sgv = sg[:, :].rearrange("p (b gt j) -> p b gt j", gt=4, j=18)
                nc.scalar.activation(
                    sgv[:, :, :, :], psv[:, :, 0:72].rearrange("p b (gt j) -> p b gt j", gt=4),
                    AF.Sigmoid)
                # cell update
                u = step.tile([128, mpg * 18], f16, name=f"u{g}", tag=f"u{g}")
                uv = u[:, :].rearrange("p (b j) -> p b j", j=18)
                nc.vector.scalar_tensor_tensor(
                    uv[:, :, :], sgv[:, :, 3, :], 0.5, sgv[:, :, 0, :],
                    op0=OP.subtract, op1=OP.mult)
                cf = step.tile([128, mpg * 18], f16, name=f"cf{g}", tag=f"cf{g}")
                cfv = cf[:, :].rearrange("p (b j) -> p b j", j=18)
                # runs on GPSIMD concurrently with the u-op on DVE
                nc.gpsimd.tensor_mul(cfv[:, :, :], ccv[:, :, :], sgv[:, :, 1, :])
                nc.vector.scalar_tensor_tensor(
                    ccv[:, :, :], uv[:, :, :], 2.0, cfv[:, :, :],
                    op0=OP.mult, op1=OP.add)
                th = step.tile([128, mpg * 18], f16, name=f"th{g}", tag=f"th{g}")
                thv = th[:, :].rearrange("p (b j) -> p b j", j=18)
                nc.scalar.activation(th[:, :], cc[:, :], AF.Tanh)
                nc.vector.tensor_mul(hv[:, :, 0:18], thv[:, :, :], sgv[:, :, 2, :])
                if tl == 0:
                    # layer-1 stream starts one step later
                    nc.vector.memset(hv[:, :, 9:18], 0.0)
                    nc.vector.memset(ccv[:, :, 9:18], 0.0)
            if tl > 0:
                # store h1 for blocks whose local output index is valid;
                # merged across groups and run on the (otherwise idle)
                # GPSIMD engine — off the recurrence critical path
                runs = []  # (bstart, bend, tt)
                cur = None
                for b in range(nblk):
                    tt = tl - 1 - blk_off(b)
                    key = tt if 0 <= tt < L else None
                    if cur is None or key != cur[2]:
                        if cur is not None and cur[2] is not None:
                            runs.append(cur)
                        cur = [b, b + 1, key]
                    else:
                        cur[1] = b + 1
                if cur is not None and cur[2] is not None:
                    runs.append(cur)
                for (bs, be, tt) in runs:
                    nc.gpsimd.tensor_copy(
                        histv[:, bs:be, tt, :],
                        hinv_all[:, bs:be, 9:18])

        # ---------------- epilogue: BN affine + relu + mean over encoders ----
        bnp = pools["bn"]
        for b in range(nblk):
            ci, half = blk_ci(b), b % halves
            hb = histv[:, b, :, :]                       # [128, L, 9]
            s9t = bnp.tile([128, L * 9], f16, name="s9t", tag="s9t")
            b9t = bnp.tile([128, L * 9], f16, name="b9t", tag="b9t")
            nc.sync.dma_start(
                out=s9t[:, :],
                in_=s9_ext[ci * L * 9:(ci + 1) * L * 9].unsqueeze(0).broadcast_to([128, L * 9]))
            nc.sync.dma_start(
                out=b9t[:, :],
                in_=b9_ext[ci * L * 9:(ci + 1) * L * 9].unsqueeze(0).broadcast_to([128, L * 9]))
            m1 = epi.tile([128, L * 9], f16, name="m1", tag="m1")
            m1v = m1[:, :].rearrange("p (t2 j) -> p t2 j", j=9)
            nc.gpsimd.tensor_mul(m1[:, :], hb.rearrange("p t2 j -> p (t2 j)"), s9t[:, :])
            z = epi.tile([128, L * 9], f16, name="z", tag="z")
            nc.vector.tensor_add(z[:, :], m1[:, :], b9t[:, :])
            z2 = epi.tile([128, L * 9], f16, name="z2", tag="z2")
            # relu on DVE (tensor_scalar 4x mode on contiguous fp16) — keeps
            # the epilogue off the busier ScalarE
            nc.vector.tensor_scalar_max(z2[:, :], z[:, :], 0.0)
            zv = z2[:, :].rearrange("p (t2 e k) -> p t2 e k", e=3, k=3)
            s1 = epi.tile([128, L * 3], f16, name="s1", tag="s1")
            s1v = s1[:, :].rearrange("p (t2 k) -> p t2 k", k=3)
            nc.gpsimd.tensor_add(s1v[:, :, :], zv[:, :, 0, :], zv[:, :, 1, :])
            ob = pools["outb"].tile([128, 3 * L], f32, name="ob", tag="ob")
            obv = ob[:, :].rearrange("p (k t2) -> p t2 k", k=3)
            nc.gpsimd.tensor_add(obv[:, :, :], s1v[:, :, :], zv[:, :, 2, :])
            nc.sync.dma_start(
                out=out_ext[half * 128:(half + 1) * 128, :, ci * L:(ci + 1) * L],
                in_=ob[:, :].rearrange("p (k t2) -> p k t2", k=3))
        ctx.close()
    nc.compile()   # bacc lowering: splits multi-sem waits, regalloc, fusion
    return nc


_BUILT = {}


def _get_built(cfg):
    key = tuple(sorted(cfg.items()))
    if key not in _BUILT:
        _BUILT[key] = _build_nc(cfg)
    return _BUILT[key]


def _host_pack(inp, cfg):
    """Build per-core input maps from full inputs."""
    spc, t = cfg['spc'], cfg['t']
    x = np.asarray(inp['x'], np.float32)
    wrep = _pack_weights({k: np.asarray(v, np.float32) for k, v in inp.items()
                          if k.startswith(('W_', 'b_'))})
    s = (np.asarray(inp['bn_gamma'], np.float32)
         / np.sqrt(np.asarray(inp['bn_var'], np.float32) + 1e-5))
    s3 = (s / 3.0).astype(np.float32)
    b3 = ((np.asarray(inp['bn_beta'], np.float32)
           - np.asarray(inp['bn_mean'], np.float32) * s) / 3.0).astype(np.float32)
    s9 = np.repeat(s3, 9).astype(np.float16)
    b9 = np.repeat(b3, 9).astype(np.float16)
    # stream s: 0..B-1 acc (ch 0:3), B..2B-1 gyr (ch 3:6)
    xs_all = np.concatenate([x[:, 0:3, :], x[:, 3:6, :]], axis=0).astype(np.float16)
    ncores = (2 * B) // spc
    in_maps = []
    for k in range(ncores):
        in_maps.append({
            "xs": np.ascontiguousarray(xs_all[k * spc:(k + 1) * spc]),
            "wrep": wrep, "s9": s9, "b9": b9,
        })
    return in_maps


def _unpack(results, cfg):
    spc, t = cfg['spc'], cfg['t']
    ncores = (2 * B) // spc
    out = np.empty((B, 6, T), np.float32)
    for k in range(ncores):
        r = np.asarray(results[k]["out"], np.float32)   # [spc, 3, t]
        s0 = k * spc
        for j in range(spc):
            s = s0 + j
            if s < B:
                out[s, 0:3, :] = r[j]
            else:
                out[s - B, 3:6, :] = r[j]
    return out


def _run(inp, trace=False):
    from concourse.bass_utils import run_bass_kernel_spmd
    cfg = CFG_FULL
    nc = _get_built(cfg)
    in_maps = _host_pack(inp, cfg)
    core_ids = list(range(len(in_maps)))
    res = run_bass_kernel_spmd(nc, in_maps, core_ids, trace=trace)
    out = _unpack(res.results, cfg)
    return out, res


def kernel(**inputs):
    out, _ = _run(inputs, trace=False)
    return out
